# revision 22
# baseline (speedup 1.0000x reference)
"""Trainium2 Bass kernel for nn_A2Attention (B=2, S=4096, H=1024, NH=16, hd=64).

Sharding: 8 cores = data-parallel over batch (2) x tensor-parallel over heads (4
groups of 4 heads). Each core computes QKV projection for its 4 heads, RMSNorm +
RoPE on Q/K, causal flash attention in transposed-score layout, and a partial
row-parallel o_proj output [4096, 1024]; the host sums the 4 partials per batch.

Self-contained: hardcodes shapes and builds/compiles the NEFF on first call.
"""

import os
import sys

for _p in ("/root/.axon_site", "/root/.axon_site/_ro/trn_rl_repo",
           "/root/.axon_site/_ro/pypackages"):
    if _p not in sys.path and os.path.isdir(_p):
        sys.path.insert(0, _p)

import numpy as np
import ml_dtypes

BF16 = ml_dtypes.bfloat16

H = 1024
NH = 16
HD = 64
NCORES = 8
HEADS_PER_CORE = 4
EPS = 1e-6

# All ACT functions used here (Exp, Ln, Copy) live in this one table set;
# restricting the candidate list makes the act-table-load pass emit a single
# load instead of thrashing between per-function canonical sets.
_ACT_SET = "natural_log_exp_and_others"


def _patch_act_tables():
    from concourse import bacc, hw_specs
    if getattr(bacc, "_act_tables_patched", False):
        return
    orig = hw_specs.get_activation_tables

    def filtered(arch):
        full = orig(arch)
        if _ACT_SET not in full:
            return full
        # Keep dict order/indices (act_func_set_id is positional); empty the
        # other sets so the load pass can only ever pick _ACT_SET.
        return {k: (v if k == _ACT_SET else type(v)())
                for k, v in full.items()}

    bacc.get_activation_tables = filtered
    bacc._act_tables_patched = True
    if os.environ.get("KERNEL_LDW_OPT", "0") == "1":
        from concourse import bass_utils as _bu
        _orig_rc = _bu.run_command

        def _rc(cmd, **kw):
            if isinstance(cmd, list):
                cmd = ["--enable-ldw-opt=true" if c == "--enable-ldw-opt=false"
                       else c for c in cmd]
            return _orig_rc(cmd, **kw)

        _bu.run_command = _rc


def build(S=4096):
    """Build the per-core Bacc graph (SPMD: same graph on all 8 cores)."""
    import concourse.mybir as mybir
    from concourse import bacc, tile

    _patch_act_tables()
    dt = mybir.dt
    AF = mybir.ActivationFunctionType
    NSC = S // 512          # s-chunks of 512
    NST = S // 128          # s-tiles of 128
    HT = H // 128           # h-tiles (contraction) = 8

    nc = bacc.Bacc("TRN2", target_bir_lowering=False)

    xt_d = nc.declare_dram_parameter("xt", [H, S], dt.bfloat16, isOutput=False)
    wq_d = nc.declare_dram_parameter("wq", [H, 256], dt.bfloat16, isOutput=False)
    wk_d = nc.declare_dram_parameter("wk", [H, 256], dt.bfloat16, isOutput=False)
    wv_d = nc.declare_dram_parameter("wv", [H, 256], dt.bfloat16, isOutput=False)
    wo_d = nc.declare_dram_parameter("wo", [256, H], dt.bfloat16, isOutput=False)
    cos_d = nc.declare_dram_parameter("cos2", [128, S], dt.bfloat16, isOutput=False)
    sin_d = nc.declare_dram_parameter("sin2", [128, S], dt.bfloat16, isOutput=False)
    gq_d = nc.declare_dram_parameter("gq", [128, 1], dt.float32, isOutput=False)
    gk_d = nc.declare_dram_parameter("gk", [128, 1], dt.float32, isOutput=False)
    out_d = nc.declare_dram_parameter("out", [S, H], dt.float32, isOutput=True)

    with tile.TileContext(nc) as tc:
        with (
            tc.tile_pool(name="const", bufs=1) as cpool,
            tc.tile_pool(name="xtp", bufs=16) as xtp,
            tc.tile_pool(name="qk", bufs=4) as qkpool,
            tc.tile_pool(name="otp", bufs=2) as otpool,
            tc.tile_pool(name="ptp", bufs=4) as ptpool,
            tc.tile_pool(name="tmp", bufs=2) as tmp,
            tc.tile_pool(name="ps", bufs=4, space="PSUM") as psp,
            tc.tile_pool(name="stp", bufs=2, space="PSUM") as stp,
        ):
            # ---- constants -------------------------------------------------
            wq_sb = cpool.tile([128, HT * 256], dt.bfloat16)
            wk_sb = cpool.tile([128, HT * 256], dt.bfloat16)
            wv_sb = cpool.tile([128, HT * 256], dt.bfloat16)
            for ht in range(HT):
                nc.sync.dma_start(out=wq_sb[:, ht * 256:(ht + 1) * 256],
                                  in_=wq_d[ht * 128:(ht + 1) * 128, :])
                nc.sync.dma_start(out=wk_sb[:, ht * 256:(ht + 1) * 256],
                                  in_=wk_d[ht * 128:(ht + 1) * 128, :])
                nc.sync.dma_start(out=wv_sb[:, ht * 256:(ht + 1) * 256],
                                  in_=wv_d[ht * 128:(ht + 1) * 128, :])
            wo_sb = cpool.tile([128, 2 * H], dt.bfloat16)
            nc.sync.dma_start(out=wo_sb[:, 0:H], in_=wo_d[0:128, :])
            nc.sync.dma_start(out=wo_sb[:, H:2 * H], in_=wo_d[128:256, :])
            cos_sb = cpool.tile([128, S], dt.bfloat16)
            sin_sb = cpool.tile([128, S], dt.bfloat16)
            nc.sync.dma_start(out=cos_sb[:], in_=cos_d[:])
            nc.sync.dma_start(out=sin_sb[:], in_=sin_d[:])
            gq_sb = cpool.tile([128, 1], dt.float32)
            gk_sb = cpool.tile([128, 1], dt.float32)
            nc.sync.dma_start(out=gq_sb[:], in_=gq_d[:])
            nc.sync.dma_start(out=gk_sb[:], in_=gk_d[:])
            # causal mask strip: strip[kk, x] = 1 if x >= kk + 384 else 0
            strip = cpool.tile([128, 896], dt.bfloat16)
            nc.gpsimd.memset(strip[:], 1.0)
            nc.gpsimd.affine_select(
                out=strip[:], in_=strip[:],
                compare_op=mybir.AluOpType.is_ge, fill=0.0,
                base=-384, pattern=[[1, 896]], channel_multiplier=-1)
            ones2 = cpool.tile([128, 33], dt.bfloat16)
            nc.gpsimd.memset(ones2[:], 0.0)
            nc.gpsimd.memset(ones2[0:64, 0:1], 1.0)
            nc.gpsimd.memset(ones2[64:128, 32:33], 1.0)
            epsb = cpool.tile([128, 1], dt.float32)
            nc.gpsimd.memset(epsb[:], EPS)

            # ---- V projection into [V_h | ones | 0...] lhsT blocks ---------
            # vzbig block (st, hl) at col st*512+hl*128: cols 0:64 = V dims of
            # head hl, col 64 = ones (gives the softmax denominator as row 64
            # of the fused attention*V matmul), cols 65:128 = zeros.
            vzbig = cpool.tile([128, NST * 512], dt.bfloat16)
            vz3 = vzbig[:].rearrange("p (b c) -> p b c", c=128)
            nc.gpsimd.memset(vz3[:, :, 64:65], 1.0)
            nc.gpsimd.memset(vz3[:, :, 65:128], 0.0)
            for st in range(NST):
                if st % 8 == 0:
                    xts = []
                    for ht in range(HT):
                        xt_t = xtp.tile([128, 1024], dt.bfloat16, tag="xt")
                        w = min(1024, S - (st // 8) * 1024)
                        nc.sync.dma_start(
                            out=xt_t[:, 0:w],
                            in_=xt_d[ht * 128:(ht + 1) * 128,
                                     (st // 8) * 1024:(st // 8) * 1024 + w])
                        xts.append(xt_t)
                v_ps = psp.tile([128, 256], dt.float32, tag="ps")
                for ht in range(HT):
                    nc.tensor.matmul(
                        v_ps[:],
                        xts[ht][:, (st % 8) * 128:(st % 8 + 1) * 128],
                        wv_sb[:, ht * 256:(ht + 1) * 256],
                        start=(ht == 0), stop=(ht == HT - 1))
                vdst = vzbig[:, st * 512:(st + 1) * 512].rearrange(
                    "p (h c) -> p h c", c=128)[:, :, 0:64]
                vsrc = v_ps[:].rearrange("p (h c) -> p h c", c=64)
                nc.vector.tensor_copy(vdst, vsrc)

            # ---- stage A: Q^T/K^T projection + rmsnorm + rope --------------
            hats = []   # [pair] -> (qhat, khat) sbuf [128, S]
            for p in range(2):
                qhat = qkpool.tile([128, S], dt.bfloat16, tag="qhat")
                khat = qkpool.tile([128, S], dt.bfloat16, tag="qhat")
                hats.append((qhat, khat))
                for sc in range(NSC):
                    if sc % 2 == 0:
                        xts = []
                        for ht in range(HT):
                            xt_t = xtp.tile([128, 1024], dt.bfloat16, tag="xt")
                            w = min(1024, S - sc * 512)
                            nc.sync.dma_start(
                                out=xt_t[:, 0:w],
                                in_=xt_d[ht * 128:(ht + 1) * 128,
                                         sc * 512:sc * 512 + w])
                            xts.append(xt_t)
                    xo = (sc % 2) * 512
                    for w_sb, hat, g_sb in ((wq_sb, qhat, gq_sb),
                                            (wk_sb, khat, gk_sb)):
                        qt_ps = psp.tile([128, 512], dt.float32, tag="ps")
                        for ht in range(HT):
                            nc.tensor.matmul(
                                qt_ps[:],
                                w_sb[:, ht * 256 + 128 * p: ht * 256 + 128 * (p + 1)],
                                xts[ht][:, xo:xo + 512],
                                start=(ht == 0), stop=(ht == HT - 1))
                        # rstd = exp(-0.5*ln(mean(q^2)+eps)) per head
                        qsq = tmp.tile([128, 512], dt.bfloat16, tag="qsq")
                        nc.scalar.activation(qsq[:], qt_ps[:], AF.Square)
                        ssq = stp.tile([33, 512], dt.float32, tag="st")
                        nc.tensor.matmul(ssq[:], ones2[:], qsq[:],
                                         start=True, stop=True)
                        rln_a = tmp.tile([1, 512], dt.float32, tag="rln_a", bufs=1)
                        rln_b = tmp.tile([1, 512], dt.float32, tag="rln_b", bufs=1)
                        nc.scalar.activation(rln_a[:], ssq[0:1, :], AF.Ln,
                                             bias=epsb[0:1, :], scale=1.0 / HD)
                        nc.scalar.activation(rln_b[:], ssq[32:33, :], AF.Ln,
                                             bias=epsb[0:1, :], scale=1.0 / HD)
                        rstd_a = tmp.tile([1, 512], dt.bfloat16, tag="rstd_a", bufs=1)
                        rstd_b = tmp.tile([1, 512], dt.bfloat16, tag="rstd_b", bufs=1)
                        nc.scalar.activation(rstd_a[:], rln_a[:], AF.Exp,
                                             scale=-0.5)
                        nc.scalar.activation(rstd_b[:], rln_b[:], AF.Exp,
                                             scale=-0.5)
                        rb = tmp.tile([128, 512], dt.bfloat16, tag="rb")
                        rbb = tmp.tile([64, 512], dt.bfloat16, tag="rbb")
                        nc.gpsimd.partition_broadcast(rb[0:64, :], rstd_a[:])
                        nc.gpsimd.partition_broadcast(rbb[:], rstd_b[:])
                        nc.sync.dma_start(out=rb[64:128, :], in_=rbb[:])
                        # gamma fold + rope + rstd apply
                        qg = tmp.tile([128, 512], dt.bfloat16, tag="qg")
                        nc.vector.tensor_scalar_mul(qg[:], qt_ps[:], g_sb[:])
                        t1 = tmp.tile([128, 512], dt.bfloat16, tag="t1")
                        nc.vector.tensor_mul(
                            t1[:], qg[:], cos_sb[:, sc * 512:(sc + 1) * 512])
                        qs = tmp.tile([128, 512], dt.bfloat16, tag="qs")
                        nc.gpsimd.dma_start(out=qs[0:32, :], in_=qg[32:64, :])
                        nc.gpsimd.dma_start(out=qs[32:64, :], in_=qg[0:32, :])
                        nc.sync.dma_start(out=qs[64:96, :], in_=qg[96:128, :])
                        nc.sync.dma_start(out=qs[96:128, :], in_=qg[64:96, :])
                        t2 = tmp.tile([128, 512], dt.bfloat16, tag="t2")
                        nc.vector.tensor_mul(
                            t2[:], qs[:], sin_sb[:, sc * 512:(sc + 1) * 512])
                        nc.vector.tensor_add(t1[:], t1[:], t2[:])
                        nc.vector.tensor_mul(hat[:, sc * 512:(sc + 1) * 512],
                                             t1[:], rb[:])

            # ---- stage B: causal flash attention, pairs interleaved --------
            ot_tiles = [otpool.tile([128, S], dt.bfloat16, tag="ot",
                                    name=f"ot{i}") for i in range(2)]
            for qc in range(NSC):
                nkt = 4 * (qc + 1)
                avs = [[psp.tile([128, 512], dt.float32, tag="ps",
                                 name=f"av{qc}_{i}_{j}") for j in range(2)]
                       for i in range(2)]
                for kt in range(nkt):
                    for p in range(2):
                        qhat, khat = hats[p]
                        avA, avB = avs[p]
                        st2 = stp.tile([128, 1024], dt.float32, tag="st")
                        nc.tensor.matmul(
                            st2[:, 0:512],
                            khat[0:64, kt * 128:(kt + 1) * 128],
                            qhat[0:64, qc * 512:(qc + 1) * 512],
                            start=True, stop=True, tile_position=(0, 0))
                        nc.tensor.matmul(
                            st2[:, 512:1024],
                            khat[64:128, kt * 128:(kt + 1) * 128],
                            qhat[64:128, qc * 512:(qc + 1) * 512],
                            start=True, stop=True, tile_position=(64, 0))
                        pt = ptpool.tile([128, 1024], dt.bfloat16, tag="pt")
                        nc.scalar.activation(pt[:], st2[:], AF.Exp, scale=0.125)
                        t = kt - 4 * qc
                        if t >= 0:
                            msl = strip[:, 384 - 128 * t: 896 - 128 * t]
                            nc.vector.tensor_mul(pt[:, 0:512], pt[:, 0:512], msl)
                            nc.vector.tensor_mul(pt[:, 512:1024],
                                                 pt[:, 512:1024], msl)
                        nc.tensor.matmul(
                            avA[:], vzbig[:, kt * 512 + 256 * p:
                                          kt * 512 + 256 * p + 128],
                            pt[:, 0:512],
                            start=(kt == 0), stop=(kt == nkt - 1))
                        nc.tensor.matmul(
                            avB[:], vzbig[:, kt * 512 + 256 * p + 128:
                                          kt * 512 + 256 * p + 256],
                            pt[:, 512:1024],
                            start=(kt == 0), stop=(kt == nkt - 1))
                for p in range(2):
                    avA, avB = avs[p]
                    # free PSUM fast: pull Z rows + out^T rows to SBUF, then
                    # normalize off-PSUM (PE stalls >3.4us re-throttle HAM).
                    zcp_a = tmp.tile([1, 512], dt.float32, tag="zcp_a", bufs=1)
                    zcp_b = tmp.tile([1, 512], dt.float32, tag="zcp_b", bufs=1)
                    nc.scalar.copy(zcp_a[:], avA[64:65, :])
                    nc.scalar.copy(zcp_b[:], avB[64:65, :])
                    avc_a = tmp.tile([64, 512], dt.float32, tag="avc_a")
                    avc_b = tmp.tile([64, 512], dt.float32, tag="avc_b")
                    nc.vector.tensor_copy(avc_a[:], avA[0:64, :])
                    nc.vector.tensor_copy(avc_b[:], avB[0:64, :])
                    rz_a = tmp.tile([1, 512], dt.float32, tag="rz_a", bufs=1)
                    rz_b = tmp.tile([1, 512], dt.float32, tag="rz_b", bufs=1)
                    nc.vector.reciprocal_approx_fast(rz_a[:], zcp_a[:])
                    nc.vector.reciprocal_approx_fast(rz_b[:], zcp_b[:])
                    rzb_a = tmp.tile([64, 512], dt.float32, tag="rzb_a")
                    rzb_b = tmp.tile([64, 512], dt.float32, tag="rzb_b")
                    nc.gpsimd.partition_broadcast(rzb_a[:], rz_a[:])
                    nc.gpsimd.partition_broadcast(rzb_b[:], rz_b[:])
                    nc.vector.tensor_mul(
                        ot_tiles[p][0:64, qc * 512:(qc + 1) * 512],
                        avc_a[:], rzb_a[:])
                    otb = tmp.tile([64, 512], dt.bfloat16, tag="otb")
                    nc.vector.tensor_mul(otb[:], avc_b[:], rzb_b[:])
                    nc.gpsimd.dma_start(
                        out=ot_tiles[p][64:128, qc * 512:(qc + 1) * 512],
                        in_=otb[:])
                # o_proj for the previous qc's s-tiles (their ot slices are
                # long finished, so these MMs never stall the PE stream)
                for st in ([] if qc == 0 else range(4 * (qc - 1), 4 * qc)):
                    for ec in range(2):
                        o_ps = psp.tile([128, 512], dt.float32, tag="ps")
                        for ct in range(2):
                            nc.tensor.matmul(
                                o_ps[:],
                                ot_tiles[ct][:, st * 128:(st + 1) * 128],
                                wo_sb[:, ct * H + ec * 512: ct * H + ec * 512 + 512],
                                start=(ct == 0), stop=(ct == 1))
                        o_sb = tmp.tile([128, 512], dt.float32, tag="osb")
                        nc.vector.tensor_copy(o_sb[:], o_ps[:])
                        nc.sync.dma_start(
                            out=out_d[st * 128:(st + 1) * 128,
                                      ec * 512:(ec + 1) * 512],
                            in_=o_sb[:])

            # last qc's s-tiles
            for st in range(S // 128 - 4, S // 128):
                for ec in range(2):
                    o_ps = psp.tile([128, 512], dt.float32, tag="ps")
                    for ct in range(2):
                        nc.tensor.matmul(
                            o_ps[:],
                            ot_tiles[ct][:, st * 128:(st + 1) * 128],
                            wo_sb[:, ct * H + ec * 512: ct * H + ec * 512 + 512],
                            start=(ct == 0), stop=(ct == 1))
                    o_sb = tmp.tile([128, 512], dt.float32, tag="osb")
                    nc.vector.tensor_copy(o_sb[:], o_ps[:])
                    nc.sync.dma_start(
                        out=out_d[st * 128:(st + 1) * 128, ec * 512:(ec + 1) * 512],
                        in_=o_sb[:])

    nc.finalize()
    return nc


def host_prep(hidden_states, rope_cos, rope_sin, W_qkv, W_o, gamma_q, gamma_k, S):
    """Build the 8 per-core input maps (bf16)."""
    hidden_states = np.asarray(hidden_states, np.float32)
    rope_cos = np.asarray(rope_cos, np.float32)
    rope_sin = np.asarray(rope_sin, np.float32)
    W_qkv = np.asarray(W_qkv, np.float32)
    W_o = np.asarray(W_o, np.float32)
    gamma_q = np.asarray(gamma_q, np.float32)
    gamma_k = np.asarray(gamma_k, np.float32)

    cos_t = np.ascontiguousarray(rope_cos[0].T)  # [64, S]
    sin_t = np.ascontiguousarray(rope_sin[0].T)
    sgn = np.where(np.arange(HD) < HD // 2, -1.0, 1.0).astype(np.float32)
    cos2 = np.concatenate([cos_t, cos_t], 0).astype(BF16)
    sin2 = np.concatenate([sgn[:, None] * sin_t] * 2, 0).astype(BF16)
    gq = np.concatenate([gamma_q, gamma_q], 0).astype(np.float32)[:, None]
    gk = np.concatenate([gamma_k, gamma_k], 0).astype(np.float32)[:, None]

    in_maps = []
    for core in range(NCORES):
        b, g = core // 4, core % 4
        h0 = g * HEADS_PER_CORE * HD  # column offset, 256 per group
        in_maps.append({
            "xt": np.ascontiguousarray(hidden_states[b].T).astype(BF16),
            "wq": W_qkv[:, h0:h0 + 256].astype(BF16),
            "wk": W_qkv[:, H + h0:H + h0 + 256].astype(BF16),
            "wv": W_qkv[:, 2 * H + h0:2 * H + h0 + 256].astype(BF16),
            "wo": W_o[h0:h0 + 256, :].astype(BF16),
            "cos2": cos2, "sin2": sin2, "gq": gq, "gk": gk,
        })
    return in_maps


_NC_CACHE = {}


def run(inputs, S=4096, trace=False):
    from concourse.bass_utils import run_bass_kernel_spmd
    if S not in _NC_CACHE:
        _NC_CACHE[S] = build(S)
    nc = _NC_CACHE[S]
    in_maps = host_prep(S=S, **inputs)
    res = run_bass_kernel_spmd(nc, in_maps, list(range(NCORES)), trace=trace)
    B = 2
    out = np.zeros((B, S, H), np.float32)
    for b in range(B):
        acc = res.results[4 * b]["out"].astype(np.float32)
        for g in range(1, 4):
            acc = acc + res.results[4 * b + g]["out"]
        out[b] = acc
    return out, res


def kernel(**inputs):
    out, _ = run(inputs, S=4096, trace=False)
    return out


# revision 24
# speedup vs baseline: 1.1924x; 1.1924x over previous
"""Trainium2 Bass kernel for nn_A2Attention (B=2, S=4096, H=1024, NH=16, hd=64).

Sharding: 8 cores = data-parallel over batch (2) x tensor-parallel over heads (4
groups of 4 heads). Each core computes QKV projection for its 4 heads, RMSNorm +
RoPE on Q/K, causal flash attention in transposed-score layout, and a partial
row-parallel o_proj output [4096, 1024]; the host sums the 4 partials per batch.

Self-contained: hardcodes shapes and builds/compiles the NEFF on first call.
"""

import os
import sys

for _p in ("/root/.axon_site", "/root/.axon_site/_ro/trn_rl_repo",
           "/root/.axon_site/_ro/pypackages"):
    if _p not in sys.path and os.path.isdir(_p):
        sys.path.insert(0, _p)

import numpy as np
import ml_dtypes

BF16 = ml_dtypes.bfloat16

H = 1024
NH = 16
HD = 64
NCORES = 8
HEADS_PER_CORE = 4
EPS = 1e-6

# All ACT functions used here (Exp, Ln, Copy) live in this one table set;
# restricting the candidate list makes the act-table-load pass emit a single
# load instead of thrashing between per-function canonical sets.
_ACT_SET = "natural_log_exp_and_others"


def _patch_act_tables():
    from concourse import bacc, hw_specs
    if getattr(bacc, "_act_tables_patched", False):
        return
    orig = hw_specs.get_activation_tables

    def filtered(arch):
        full = orig(arch)
        if _ACT_SET not in full:
            return full
        # Keep dict order/indices (act_func_set_id is positional); empty the
        # other sets so the load pass can only ever pick _ACT_SET.
        return {k: (v if k == _ACT_SET else type(v)())
                for k, v in full.items()}

    bacc.get_activation_tables = filtered
    bacc._act_tables_patched = True
    if os.environ.get("KERNEL_LDW_OPT", "0") == "1":
        from concourse import bass_utils as _bu
        _orig_rc = _bu.run_command

        def _rc(cmd, **kw):
            if isinstance(cmd, list):
                cmd = ["--enable-ldw-opt=true" if c == "--enable-ldw-opt=false"
                       else c for c in cmd]
            return _orig_rc(cmd, **kw)

        _bu.run_command = _rc


def build(S=4096):
    """Build the per-core Bacc graph (SPMD: same graph on all 8 cores)."""
    import concourse.mybir as mybir
    from concourse import bacc, tile

    _patch_act_tables()
    dt = mybir.dt
    AF = mybir.ActivationFunctionType
    NSC = S // 512          # s-chunks of 512
    NST = S // 128          # s-tiles of 128
    HT = H // 128           # h-tiles (contraction) = 8

    nc = bacc.Bacc("TRN2", target_bir_lowering=False)

    xt_d = nc.declare_dram_parameter("xt", [H, S], dt.bfloat16, isOutput=False)
    wq_d = nc.declare_dram_parameter("wq", [H, 256], dt.bfloat16, isOutput=False)
    wk_d = nc.declare_dram_parameter("wk", [H, 256], dt.bfloat16, isOutput=False)
    wv_d = nc.declare_dram_parameter("wv", [H, 256], dt.bfloat16, isOutput=False)
    wo_d = nc.declare_dram_parameter("wo", [256, H], dt.bfloat16, isOutput=False)
    cos_d = nc.declare_dram_parameter("cos2", [128, S], dt.bfloat16, isOutput=False)
    sin_d = nc.declare_dram_parameter("sin2", [128, S], dt.bfloat16, isOutput=False)
    gq_d = nc.declare_dram_parameter("gq", [128, 1], dt.float32, isOutput=False)
    gk_d = nc.declare_dram_parameter("gk", [128, 1], dt.float32, isOutput=False)
    out_d = nc.declare_dram_parameter("out", [S, H], dt.float32, isOutput=True)

    with tile.TileContext(nc) as tc:
        with (
            tc.tile_pool(name="const", bufs=1) as cpool,
            tc.tile_pool(name="xtp", bufs=12) as xtp,
            tc.tile_pool(name="qk", bufs=4) as qkpool,
            tc.tile_pool(name="otp", bufs=2) as otpool,
            tc.tile_pool(name="ptp", bufs=6) as ptpool,
            tc.tile_pool(name="tmp", bufs=2) as tmp,
            tc.tile_pool(name="ps", bufs=4, space="PSUM") as psp,
            tc.tile_pool(name="stp", bufs=2, space="PSUM") as stp,
        ):
            # ---- constants -------------------------------------------------
            wq_sb = cpool.tile([128, HT * 256], dt.bfloat16)
            wk_sb = cpool.tile([128, HT * 256], dt.bfloat16)
            wv_sb = cpool.tile([128, HT * 256], dt.bfloat16)
            for ht in range(HT):
                nc.sync.dma_start(out=wq_sb[:, ht * 256:(ht + 1) * 256],
                                  in_=wq_d[ht * 128:(ht + 1) * 128, :])
                nc.sync.dma_start(out=wk_sb[:, ht * 256:(ht + 1) * 256],
                                  in_=wk_d[ht * 128:(ht + 1) * 128, :])
                nc.sync.dma_start(out=wv_sb[:, ht * 256:(ht + 1) * 256],
                                  in_=wv_d[ht * 128:(ht + 1) * 128, :])
            wo_sb = cpool.tile([128, 2 * H], dt.bfloat16)
            nc.sync.dma_start(out=wo_sb[:, 0:H], in_=wo_d[0:128, :])
            nc.sync.dma_start(out=wo_sb[:, H:2 * H], in_=wo_d[128:256, :])
            cos_sb = cpool.tile([128, S], dt.bfloat16)
            sin_sb = cpool.tile([128, S], dt.bfloat16)
            nc.sync.dma_start(out=cos_sb[:], in_=cos_d[:])
            nc.sync.dma_start(out=sin_sb[:], in_=sin_d[:])
            gq_sb = cpool.tile([128, 1], dt.float32)
            gk_sb = cpool.tile([128, 1], dt.float32)
            nc.sync.dma_start(out=gq_sb[:], in_=gq_d[:])
            nc.sync.dma_start(out=gk_sb[:], in_=gk_d[:])
            # causal mask strip: strip[kk, x] = 1 if x >= kk + 384 else 0
            strip = cpool.tile([128, 896], dt.bfloat16)
            nc.gpsimd.memset(strip[:], 1.0)
            nc.gpsimd.affine_select(
                out=strip[:], in_=strip[:],
                compare_op=mybir.AluOpType.is_ge, fill=0.0,
                base=-384, pattern=[[1, 896]], channel_multiplier=-1)
            ones2 = cpool.tile([128, 33], dt.bfloat16)
            nc.gpsimd.memset(ones2[:], 0.0)
            nc.gpsimd.memset(ones2[0:64, 0:1], 1.0)
            nc.gpsimd.memset(ones2[64:128, 32:33], 1.0)
            epsb = cpool.tile([128, 1], dt.float32)
            nc.gpsimd.memset(epsb[:], EPS)

            # ---- V projection into [V_h | ones | 0...] lhsT blocks ---------
            # vzbig block (st, hl) at col st*512+hl*128: cols 0:64 = V dims of
            # head hl, col 64 = ones (gives the softmax denominator as row 64
            # of the fused attention*V matmul), cols 65:128 = zeros.
            vzbig = cpool.tile([128, NST * 512], dt.bfloat16)
            vz3 = vzbig[:].rearrange("p (b c) -> p b c", c=128)
            nc.gpsimd.memset(vz3[:, :, 64:65], 1.0)
            nc.gpsimd.memset(vz3[:, :, 65:128], 0.0)
            for st in range(NST):
                if st % 8 == 0:
                    xts = []
                    for ht in range(HT):
                        xt_t = xtp.tile([128, 1024], dt.bfloat16, tag="xt")
                        w = min(1024, S - (st // 8) * 1024)
                        nc.sync.dma_start(
                            out=xt_t[:, 0:w],
                            in_=xt_d[ht * 128:(ht + 1) * 128,
                                     (st // 8) * 1024:(st // 8) * 1024 + w])
                        xts.append(xt_t)
                v_ps = psp.tile([128, 256], dt.float32, tag="ps")
                for ht in range(HT):
                    nc.tensor.matmul(
                        v_ps[:],
                        xts[ht][:, (st % 8) * 128:(st % 8 + 1) * 128],
                        wv_sb[:, ht * 256:(ht + 1) * 256],
                        start=(ht == 0), stop=(ht == HT - 1))
                vdst = vzbig[:, st * 512:(st + 1) * 512].rearrange(
                    "p (h c) -> p h c", c=128)[:, :, 0:64]
                vsrc = v_ps[:].rearrange("p (h c) -> p h c", c=64)
                nc.vector.tensor_copy(vdst, vsrc)

            # ---- stage A: Q^T/K^T projection + rmsnorm + rope --------------
            hats = []   # [pair] -> (qhat, khat) sbuf [128, S]
            for p in range(2):
                qhat = qkpool.tile([128, S], dt.bfloat16, tag="qhat")
                khat = qkpool.tile([128, S], dt.bfloat16, tag="qhat")
                hats.append((qhat, khat))
                for sc in range(NSC):
                    if sc % 2 == 0:
                        xts = []
                        for ht in range(HT):
                            xt_t = xtp.tile([128, 1024], dt.bfloat16, tag="xt")
                            w = min(1024, S - sc * 512)
                            nc.sync.dma_start(
                                out=xt_t[:, 0:w],
                                in_=xt_d[ht * 128:(ht + 1) * 128,
                                         sc * 512:sc * 512 + w])
                            xts.append(xt_t)
                    xo = (sc % 2) * 512
                    for w_sb, hat, g_sb in ((wq_sb, qhat, gq_sb),
                                            (wk_sb, khat, gk_sb)):
                        qt_ps = psp.tile([128, 512], dt.float32, tag="ps")
                        for ht in range(HT):
                            nc.tensor.matmul(
                                qt_ps[:],
                                w_sb[:, ht * 256 + 128 * p: ht * 256 + 128 * (p + 1)],
                                xts[ht][:, xo:xo + 512],
                                start=(ht == 0), stop=(ht == HT - 1))
                        # rstd = exp(-0.5*ln(mean(q^2)+eps)) per head
                        qsq = tmp.tile([128, 512], dt.bfloat16, tag="qsq")
                        nc.scalar.activation(qsq[:], qt_ps[:], AF.Square)
                        ssq = stp.tile([33, 512], dt.float32, tag="st")
                        nc.tensor.matmul(ssq[:], ones2[:], qsq[:],
                                         start=True, stop=True)
                        rln_a = tmp.tile([1, 512], dt.float32, tag="rln_a", bufs=1)
                        rln_b = tmp.tile([1, 512], dt.float32, tag="rln_b", bufs=1)
                        nc.scalar.activation(rln_a[:], ssq[0:1, :], AF.Ln,
                                             bias=epsb[0:1, :], scale=1.0 / HD)
                        nc.scalar.activation(rln_b[:], ssq[32:33, :], AF.Ln,
                                             bias=epsb[0:1, :], scale=1.0 / HD)
                        rstd_a = tmp.tile([1, 512], dt.bfloat16, tag="rstd_a", bufs=1)
                        rstd_b = tmp.tile([1, 512], dt.bfloat16, tag="rstd_b", bufs=1)
                        nc.scalar.activation(rstd_a[:], rln_a[:], AF.Exp,
                                             scale=-0.5)
                        nc.scalar.activation(rstd_b[:], rln_b[:], AF.Exp,
                                             scale=-0.5)
                        rb = tmp.tile([128, 512], dt.bfloat16, tag="rb")
                        rbb = tmp.tile([64, 512], dt.bfloat16, tag="rbb")
                        nc.gpsimd.partition_broadcast(rb[0:64, :], rstd_a[:])
                        nc.gpsimd.partition_broadcast(rbb[:], rstd_b[:])
                        nc.sync.dma_start(out=rb[64:128, :], in_=rbb[:])
                        # gamma fold + rope + rstd apply
                        qg = tmp.tile([128, 512], dt.bfloat16, tag="qg")
                        nc.vector.tensor_scalar_mul(qg[:], qt_ps[:], g_sb[:])
                        t1 = tmp.tile([128, 512], dt.bfloat16, tag="t1")
                        nc.vector.tensor_mul(
                            t1[:], qg[:], cos_sb[:, sc * 512:(sc + 1) * 512])
                        qs = tmp.tile([128, 512], dt.bfloat16, tag="qs")
                        nc.gpsimd.dma_start(out=qs[0:32, :], in_=qg[32:64, :])
                        nc.gpsimd.dma_start(out=qs[32:64, :], in_=qg[0:32, :])
                        nc.sync.dma_start(out=qs[64:96, :], in_=qg[96:128, :])
                        nc.sync.dma_start(out=qs[96:128, :], in_=qg[64:96, :])
                        t2 = tmp.tile([128, 512], dt.bfloat16, tag="t2")
                        nc.vector.tensor_mul(
                            t2[:], qs[:], sin_sb[:, sc * 512:(sc + 1) * 512])
                        nc.vector.tensor_add(t1[:], t1[:], t2[:])
                        nc.vector.tensor_mul(hat[:, sc * 512:(sc + 1) * 512],
                                             t1[:], rb[:])

            # ---- stage B: causal flash attention, pairs interleaved --------
            ot_tiles = [otpool.tile([128, S], dt.bfloat16, tag="ot",
                                    name=f"ot{i}") for i in range(2)]
            for qc in range(NSC):
                nkt = 4 * (qc + 1)
                avs = [[psp.tile([128, 512], dt.float32, tag="ps",
                                 name=f"av{qc}_{i}_{j}") for j in range(2)]
                       for i in range(2)]
                for kt in range(nkt):
                    for p in range(2):
                        qhat, khat = hats[p]
                        avA, avB = avs[p]
                        st2 = stp.tile([128, 1024], dt.float32, tag="st")
                        nc.tensor.matmul(
                            st2[:, 0:512],
                            khat[0:64, kt * 128:(kt + 1) * 128],
                            qhat[0:64, qc * 512:(qc + 1) * 512],
                            start=True, stop=True, tile_position=(0, 0))
                        nc.tensor.matmul(
                            st2[:, 512:1024],
                            khat[64:128, kt * 128:(kt + 1) * 128],
                            qhat[64:128, qc * 512:(qc + 1) * 512],
                            start=True, stop=True, tile_position=(64, 0))
                        pt = ptpool.tile([128, 1024], dt.bfloat16, tag="pt")
                        nc.scalar.activation(pt[:], st2[:], AF.Exp, scale=0.125)
                        t = kt - 4 * qc
                        if t >= 0:
                            msl = strip[:, 384 - 128 * t: 896 - 128 * t]
                            nc.vector.tensor_mul(pt[:, 0:512], pt[:, 0:512], msl)
                            nc.vector.tensor_mul(pt[:, 512:1024],
                                                 pt[:, 512:1024], msl)
                        nc.tensor.matmul(
                            avA[:], vzbig[:, kt * 512 + 256 * p:
                                          kt * 512 + 256 * p + 128],
                            pt[:, 0:512],
                            start=(kt == 0), stop=(kt == nkt - 1))
                        nc.tensor.matmul(
                            avB[:], vzbig[:, kt * 512 + 256 * p + 128:
                                          kt * 512 + 256 * p + 256],
                            pt[:, 512:1024],
                            start=(kt == 0), stop=(kt == nkt - 1))
                for p in range(2):
                    avA, avB = avs[p]
                    # free PSUM fast: pull Z rows + out^T rows to SBUF, then
                    # normalize off-PSUM (PE stalls >3.4us re-throttle HAM).
                    zcp_a = tmp.tile([1, 512], dt.float32, tag="zcp_a", bufs=1)
                    zcp_b = tmp.tile([1, 512], dt.float32, tag="zcp_b", bufs=1)
                    nc.scalar.copy(zcp_a[:], avA[64:65, :])
                    nc.scalar.copy(zcp_b[:], avB[64:65, :])
                    avc_a = tmp.tile([64, 512], dt.float32, tag="avc_a")
                    avc_b = tmp.tile([64, 512], dt.float32, tag="avc_b")
                    nc.vector.tensor_copy(avc_a[:], avA[0:64, :])
                    nc.vector.tensor_copy(avc_b[:], avB[0:64, :])
                    rz_a = tmp.tile([1, 512], dt.float32, tag="rz_a", bufs=1)
                    rz_b = tmp.tile([1, 512], dt.float32, tag="rz_b", bufs=1)
                    nc.vector.reciprocal_approx_fast(rz_a[:], zcp_a[:])
                    nc.vector.reciprocal_approx_fast(rz_b[:], zcp_b[:])
                    rzb_a = tmp.tile([64, 512], dt.float32, tag="rzb_a")
                    rzb_b = tmp.tile([64, 512], dt.float32, tag="rzb_b")
                    nc.gpsimd.partition_broadcast(rzb_a[:], rz_a[:])
                    nc.gpsimd.partition_broadcast(rzb_b[:], rz_b[:])
                    nc.vector.tensor_mul(
                        ot_tiles[p][0:64, qc * 512:(qc + 1) * 512],
                        avc_a[:], rzb_a[:])
                    otb = tmp.tile([64, 512], dt.bfloat16, tag="otb")
                    nc.vector.tensor_mul(otb[:], avc_b[:], rzb_b[:])
                    nc.gpsimd.dma_start(
                        out=ot_tiles[p][64:128, qc * 512:(qc + 1) * 512],
                        in_=otb[:])
                # o_proj for the previous qc's s-tiles (their ot slices are
                # long finished, so these MMs never stall the PE stream)
                for st in ([] if qc == 0 else range(4 * (qc - 1), 4 * qc)):
                    for ec in range(2):
                        o_ps = psp.tile([128, 512], dt.float32, tag="ps")
                        for ct in range(2):
                            nc.tensor.matmul(
                                o_ps[:],
                                ot_tiles[ct][:, st * 128:(st + 1) * 128],
                                wo_sb[:, ct * H + ec * 512: ct * H + ec * 512 + 512],
                                start=(ct == 0), stop=(ct == 1))
                        o_sb = tmp.tile([128, 512], dt.float32, tag="osb")
                        nc.vector.tensor_copy(o_sb[:], o_ps[:])
                        nc.sync.dma_start(
                            out=out_d[st * 128:(st + 1) * 128,
                                      ec * 512:(ec + 1) * 512],
                            in_=o_sb[:])

            # last qc's s-tiles
            for st in range(S // 128 - 4, S // 128):
                for ec in range(2):
                    o_ps = psp.tile([128, 512], dt.float32, tag="ps")
                    for ct in range(2):
                        nc.tensor.matmul(
                            o_ps[:],
                            ot_tiles[ct][:, st * 128:(st + 1) * 128],
                            wo_sb[:, ct * H + ec * 512: ct * H + ec * 512 + 512],
                            start=(ct == 0), stop=(ct == 1))
                    o_sb = tmp.tile([128, 512], dt.float32, tag="osb")
                    nc.vector.tensor_copy(o_sb[:], o_ps[:])
                    nc.sync.dma_start(
                        out=out_d[st * 128:(st + 1) * 128, ec * 512:(ec + 1) * 512],
                        in_=o_sb[:])

    nc.finalize()
    return nc


def host_prep(hidden_states, rope_cos, rope_sin, W_qkv, W_o, gamma_q, gamma_k, S):
    """Build the 8 per-core input maps (bf16)."""
    hidden_states = np.asarray(hidden_states, np.float32)
    rope_cos = np.asarray(rope_cos, np.float32)
    rope_sin = np.asarray(rope_sin, np.float32)
    W_qkv = np.asarray(W_qkv, np.float32)
    W_o = np.asarray(W_o, np.float32)
    gamma_q = np.asarray(gamma_q, np.float32)
    gamma_k = np.asarray(gamma_k, np.float32)

    cos_t = np.ascontiguousarray(rope_cos[0].T)  # [64, S]
    sin_t = np.ascontiguousarray(rope_sin[0].T)
    sgn = np.where(np.arange(HD) < HD // 2, -1.0, 1.0).astype(np.float32)
    cos2 = np.concatenate([cos_t, cos_t], 0).astype(BF16)
    sin2 = np.concatenate([sgn[:, None] * sin_t] * 2, 0).astype(BF16)
    gq = np.concatenate([gamma_q, gamma_q], 0).astype(np.float32)[:, None]
    gk = np.concatenate([gamma_k, gamma_k], 0).astype(np.float32)[:, None]

    in_maps = []
    for core in range(NCORES):
        b, g = core // 4, core % 4
        h0 = g * HEADS_PER_CORE * HD  # column offset, 256 per group
        in_maps.append({
            "xt": np.ascontiguousarray(hidden_states[b].T).astype(BF16),
            "wq": W_qkv[:, h0:h0 + 256].astype(BF16),
            "wk": W_qkv[:, H + h0:H + h0 + 256].astype(BF16),
            "wv": W_qkv[:, 2 * H + h0:2 * H + h0 + 256].astype(BF16),
            "wo": W_o[h0:h0 + 256, :].astype(BF16),
            "cos2": cos2, "sin2": sin2, "gq": gq, "gk": gk,
        })
    return in_maps


_NC_CACHE = {}


def run(inputs, S=4096, trace=False):
    from concourse.bass_utils import run_bass_kernel_spmd
    if S not in _NC_CACHE:
        _NC_CACHE[S] = build(S)
    nc = _NC_CACHE[S]
    in_maps = host_prep(S=S, **inputs)
    res = run_bass_kernel_spmd(nc, in_maps, list(range(NCORES)), trace=trace)
    B = 2
    out = np.zeros((B, S, H), np.float32)
    for b in range(B):
        acc = res.results[4 * b]["out"].astype(np.float32)
        for g in range(1, 4):
            acc = acc + res.results[4 * b + g]["out"]
        out[b] = acc
    return out, res


def kernel(**inputs):
    out, _ = run(inputs, S=4096, trace=False)
    return out
